# revision 40
# baseline (speedup 1.0000x reference)
"""Self-contained Trainium2 Bass kernel for the "Attentive" GNN message-passing
problem:

    x: [8192, 256] f32, attn_vectors: [4, 256] f32
    e_h = l2_normalize(attn_vectors[h] * x, axis=-1)        # [H, N, D]
    Y   = concat_h(e_h)                                     # [N, H*D]
    out = (Y @ Y.T) / H                                     # [N, N]

Strategy (8 NeuronCores, SPMD, no collectives):
  - Output rows are sharded 8 x 1024; every core receives the FULL x plus its
    own x_local row-shard as separate inputs, so the program is core-agnostic.
  - Key algebra: out[i,j] = sum_k (x*a^2*r/H)[i,k] * (x*r)[j,k] with
    r_h[n] = 1/sqrt(max(sum_d (a_h[d]*x[n,d])^2, eps)); a^2 and the 1/H are
    folded into the (small, resident) lhsT side only, so the streamed rhs
    panels need just one elementwise multiply each.
  - Everything runs in "features on partitions" layout (x^T), obtained by
    staging a bf16 copy of x in DRAM (per-panel tiles, fine-grained deps)
    and reading it back through the DMA xbar transpose.
  - Row norms are computed as transposed PE matmuls xsq^T @ a^2 so the
    max/sqrt/reciprocal chain runs in [128, 16] layout (all DVE lanes
    active); rnorm rows bounce through DRAM and come back as one batched
    broadcast DMA per panel (step-0 partition APs are legal on DRAM).
  - Matmul inputs are bf16 (PE runs f32 at quarter rate); PSUM accumulates
    f32; each panel's 8 PSUM tiles are copied into one SBUF tile and leave
    in a single 2 MB DMA.
  - DMAs are batched aggressively: the SP sequencer pays ~600 ns per
    dma_start, so the panel pipeline uses ~7 DMAs per 512-column panel.
"""

from contextlib import ExitStack

import numpy as np

N, D, H = 8192, 256, 4
NCORES = 8
NLOC = N // NCORES  # 1024 output rows per core
P = 128
PANEL = 512
NPANELS = N // PANEL  # 16
RBLK = NLOC // P  # 8 row blocks of the local output
KCH = (H * D) // P  # 8 contraction chunks of 128
CHD = D // P  # 2 chunks per head
SUB = PANEL // P  # 4 column sub-blocks per panel
EPS = 1e-12

_COMPILED = {}


def _build_bass():
    import concourse.bass as bass
    import concourse.tile as tile
    from concourse import bacc, mybir

    f32 = mybir.dt.float32
    bf16 = mybir.dt.bfloat16

    nc = bacc.Bacc(
        "TRN2",
        target_bir_lowering=False,
        debug=False,
        enable_asserts=False,
        num_devices=NCORES,
    )
    x_t = nc.dram_tensor("x", [N, D], f32, kind="ExternalInput")
    xl_t = nc.dram_tensor("x_local", [NLOC, D], f32, kind="ExternalInput")
    # Host-precomputed functions of attn_vectors (tiny):
    #   w_sq[d, c*4+h]  = attn[h, c*128+d]^2          (bf16, norm matmul rhs)
    #   asq[d, kc]      = 0.25*attn[h, c*128+d]^2     (f32, kc = h*2+c)
    ws_t = nc.dram_tensor("w_sq", [P, CHD * H], bf16, kind="ExternalInput")
    aq_t = nc.dram_tensor("asq", [P, KCH], f32, kind="ExternalInput")
    out_t = nc.dram_tensor("out", [NLOC, N], f32, kind="ExternalOutput")

    x, xl, out = x_t.ap(), xl_t.ap(), out_t.ap()

    with tile.TileContext(nc) as tc, ExitStack() as ctx:
        consts = ctx.enter_context(tc.tile_pool(name="consts", bufs=1))
        loads = ctx.enter_context(tc.tile_pool(name="loads", bufs=6))
        xtp = ctx.enter_context(tc.tile_pool(name="xtp", bufs=1))
        sq = ctx.enter_context(tc.tile_pool(name="sq", bufs=4))
        small = ctx.enter_context(tc.tile_pool(name="small", bufs=3))
        bcp = ctx.enter_context(tc.tile_pool(name="bcp", bufs=3))
        rhsp = ctx.enter_context(tc.tile_pool(name="rhsp", bufs=3))
        outp = ctx.enter_context(tc.tile_pool(name="outp", bufs=2))
        dram = ctx.enter_context(tc.tile_pool(name="dram", bufs=1, space="DRAM"))
        ps_norm = ctx.enter_context(
            tc.tile_pool(name="ps_norm", bufs=2, space="PSUM")
        )
        ps_out = ctx.enter_context(
            tc.tile_pool(name="ps_out", bufs=4, space="PSUM")
        )

        from concourse.masks import make_identity

        w_sq = consts.tile([P, CHD * H], bf16)
        nc.sync.dma_start(w_sq[:], ws_t.ap()[:])
        asq = consts.tile([P, KCH], f32)
        nc.sync.dma_start(asq[:], aq_t.ap()[:])
        ident = consts.tile([P, P], f32)
        make_identity(nc, ident[:])

        def sb_rearr(tile_ap):
            return tile_ap[:].rearrange("q (i d) -> q i d", i=SUB)

        def x_rearr(ap, row0):
            return ap[row0 : row0 + PANEL, :].rearrange(
                "(i q) d -> q i d", q=P
            )

        def prepass(src_ap, row0, xT_tile, name):
            """Load 512 source rows (one batched DMA), transpose them on the
            PE into bf16 x^T, and park this panel's rnorm in DRAM.
            The PSUM->SBUF copy after each transpose doubles as the f32->bf16
            cast."""
            xt = loads.tile([P, SUB * D], f32, tag="xload")
            nc.sync.dma_start(sb_rearr(xt), x_rearr(src_ap, row0))
            for c in range(CHD):
                tp4 = ps_norm.tile([P, PANEL], f32, tag="tp")
                for i in range(SUB):
                    nc.tensor.transpose(
                        tp4[:, i * P : (i + 1) * P],
                        xt[:, i * D + c * P : i * D + (c + 1) * P],
                        ident[:],
                    )
                nc.vector.tensor_copy(
                    xT_tile[:, c * PANEL : (c + 1) * PANEL], tp4[:]
                )
            pn = ps_norm.tile([P, SUB * H], f32, tag="pn")
            xsq = sq.tile([P, CHD * PANEL], bf16, tag="xsq")
            nc.vector.tensor_mul(xsq[:], xT_tile[:], xT_tile[:])
            for i in range(SUB):
                for c in range(CHD):
                    nc.tensor.matmul(
                        pn[:, i * H : (i + 1) * H],
                        xsq[:, c * PANEL + i * P : c * PANEL + (i + 1) * P],
                        w_sq[:, c * H : (c + 1) * H],
                        start=(c == 0),
                        stop=(c == CHD - 1),
                    )
            # eps-clamp; the input AP also permutes [q,(i h)] -> [q,(h i)]
            # so that after the PE transpose the store is contiguous.
            clamped = small.tile([P, SUB * H], f32, tag="clamped")
            nc.vector.tensor_scalar_max(
                clamped[:],
                pn[:].rearrange("q (i h) -> q h i", h=H),
                EPS,
            )
            root = small.tile([P, SUB * H], f32, tag="root")
            nc.scalar.sqrt(root[:], clamped[:])
            rnorm = small.tile([P, SUB * H], f32, tag="rnorm")
            nc.vector.reciprocal(rnorm[:], root[:])
            # [128, 16] -> [16, 128]; row j = h*4+i, so the flat DRAM tile
            # is rnorm_h[i*128+q] at offset h*512 + i*128 + q (h-major).
            pt = ps_norm.tile([SUB * H, P], f32, tag="tp")
            nc.tensor.transpose(pt[:], rnorm[:], ident[:])
            rno = small.tile([SUB * H, P], f32, tag="rno")
            nc.vector.tensor_copy(rno[:], pt[:])
            rnd = dram.tile([SUB * H, P], f32, name=name)
            nc.sync.dma_start(rnd[:], rno[:])
            return rnd

        def bcast_rnorm(rnd):
            """[128, 4*512] f32: bc[:, h*512 + n] = rnorm_h[n], one DMA."""
            bc = bcp.tile([P, H * PANEL], f32, tag="bc")
            src = bass.AP(
                rnd.tensor,
                rnd.offset,
                [[0, P], [PANEL, H], [1, PANEL]],
            )
            nc.sync.dma_start(
                bc[:].rearrange("p (h n) -> p h n", h=H), src
            )
            return bc

        # ---- all prepasses first ------------------------------------------
        # Tile's per-engine instruction order is static, so the lhsT-build
        # DVE ops (which wait on the rnorm DRAM bounce) must come AFTER every
        # prepass op or they head-of-line-block the prepass copies and starve
        # the PE of transpose work during the wait.
        lhsT = consts.tile([P, KCH * NLOC], bf16)
        xlocT = []
        lrnds = []
        for lp in range(2):
            t = consts.tile([P, CHD * PANEL], bf16, name=f"xlocT{lp}")
            xlocT.append(t)
            lrnds.append(prepass(xl, lp * PANEL, t, f"lrnd{lp}"))
        xTs = []
        rnds = []
        for p in range(NPANELS):
            t = xtp.tile([P, CHD * PANEL], bf16, name=f"xT{p}")
            xTs.append(t)
            rnds.append(prepass(x, p * PANEL, t, f"rnd{p}"))

        # ---- resident lhsT -------------------------------------------------
        for lp in range(2):
            t = xlocT[lp]
            bc = bcast_rnorm(lrnds[lp])
            for h in range(H):
                for c in range(CHD):
                    kc = h * CHD + c
                    scaled = sq.tile([P, PANEL], f32, tag="scaled")
                    nc.vector.tensor_scalar_mul(
                        scaled[:],
                        bc[:, h * PANEL : (h + 1) * PANEL],
                        asq[:, kc : kc + 1],
                    )
                    nc.vector.tensor_mul(
                        lhsT[
                            :,
                            kc * NLOC + lp * PANEL : kc * NLOC + (lp + 1) * PANEL,
                        ],
                        t[:, c * PANEL : (c + 1) * PANEL],
                        scaled[:],
                    )

        # ---- main loop over 16 column panels -------------------------------
        for p in range(NPANELS):
            bc = bcast_rnorm(rnds[p])
            rhs = rhsp.tile([P, KCH * PANEL], bf16, tag="rhs")
            # One batched multiply builds the whole Y'^T panel:
            #   rhs[:, (h*2+c)*512 + n] = xT[:, c*512 + n] * bc[:, h*512 + n]
            xT = xTs[p]
            in0 = bass.AP(
                xT.tensor,
                xT.offset,
                [list(xT.ap[0]), [0, H], [PANEL, CHD], [1, PANEL]],
            )
            in1 = bass.AP(
                bc.tensor,
                bc.offset,
                [list(bc.ap[0]), [PANEL, H], [0, CHD], [1, PANEL]],
            )
            nc.vector.tensor_tensor(
                rhs[:].rearrange("q (h c n) -> q h c n", h=H, c=CHD),
                in0,
                in1,
                mybir.AluOpType.mult,
            )

            ot = outp.tile([P, RBLK * PANEL], f32, tag="ot")
            for r in range(RBLK):
                acc = ps_out.tile([P, PANEL], f32, tag="acc")
                for kc in range(KCH):
                    nc.tensor.matmul(
                        acc[:],
                        lhsT[:, kc * NLOC + r * P : kc * NLOC + (r + 1) * P],
                        rhs[:, kc * PANEL : (kc + 1) * PANEL],
                        start=(kc == 0),
                        stop=(kc == KCH - 1),
                    )
                nc.vector.tensor_copy(
                    ot[:, r * PANEL : (r + 1) * PANEL], acc[:]
                )
                # Last panel: ship each row block as soon as it is ready so
                # the kernel tail is one small DMA, not copy-all-then-DMA.
                if p == NPANELS - 1:
                    nc.sync.dma_start(
                        out[
                            r * P : (r + 1) * P,
                            p * PANEL : (p + 1) * PANEL,
                        ],
                        ot[:, r * PANEL : (r + 1) * PANEL],
                    )
            if p != NPANELS - 1:
                nc.sync.dma_start(
                    out[:, p * PANEL : (p + 1) * PANEL].rearrange(
                        "(r q) c -> q r c", q=P
                    ),
                    ot[:].rearrange("q (r c) -> q r c", r=RBLK),
                )

    nc.compile()
    return nc


def _get_compiled():
    if "nc" not in _COMPILED:
        _COMPILED["nc"] = _build_bass()
    return _COMPILED["nc"]


def host_side_inputs(x, attn):
    """Per-core input maps (w_sq / asq are tiny host-precomputed functions
    of attn_vectors; see _build_bass)."""
    import ml_dtypes

    w_sq = np.zeros((P, CHD * H), dtype=np.float32)
    asq = np.zeros((P, KCH), dtype=np.float32)
    for c in range(CHD):
        w_sq[:, c * H : (c + 1) * H] = (attn[:, c * P : (c + 1) * P] ** 2).T
    for kc in range(KCH):
        h, c = divmod(kc, CHD)
        asq[:, kc] = 0.25 * attn[h, c * P : (c + 1) * P] ** 2
    w_sq = w_sq.astype(ml_dtypes.bfloat16)
    return [
        {
            "x": x,
            "x_local": np.ascontiguousarray(x[c * NLOC : (c + 1) * NLOC]),
            "w_sq": w_sq,
            "asq": asq,
        }
        for c in range(NCORES)
    ]


def kernel(**inputs) -> np.ndarray:
    from concourse import bass_utils

    x = np.ascontiguousarray(np.asarray(inputs["x"], dtype=np.float32))
    attn = np.ascontiguousarray(
        np.asarray(inputs["attn_vectors"], dtype=np.float32)
    )
    nc = _get_compiled()
    res = bass_utils.run_bass_kernel_spmd(
        nc, host_side_inputs(x, attn), core_ids=list(range(NCORES))
    )
    out = np.concatenate([r["out"] for r in res.results], axis=0)
    # The exact result is symmetric; the bf16 rounding errors of the two
    # triangles are independent, so symmetrizing averages them down.
    return ((out + out.T) * 0.5).astype(np.float32)


# revision 43
# speedup vs baseline: 1.0144x; 1.0144x over previous
"""Self-contained Trainium2 Bass kernel for the "Attentive" GNN message-passing
problem:

    x: [8192, 256] f32, attn_vectors: [4, 256] f32
    e_h = l2_normalize(attn_vectors[h] * x, axis=-1)        # [H, N, D]
    Y   = concat_h(e_h)                                     # [N, H*D]
    out = (Y @ Y.T) / H                                     # [N, N]

Strategy (8 NeuronCores, SPMD, no collectives):
  - Output rows are sharded 8 x 1024; every core receives the FULL x plus its
    own x_local row-shard as separate inputs, so the program is core-agnostic.
  - Key algebra: out[i,j] = sum_k (x*a^2*r/H)[i,k] * (x*r)[j,k] with
    r_h[n] = 1/sqrt(max(sum_d (a_h[d]*x[n,d])^2, eps)); a^2 and the 1/H are
    folded into the (small, resident) lhsT side only, so the streamed rhs
    panels need just one elementwise multiply each.
  - Everything runs in "features on partitions" layout (x^T), obtained by
    staging a bf16 copy of x in DRAM (per-panel tiles, fine-grained deps)
    and reading it back through the DMA xbar transpose.
  - Row norms are computed as transposed PE matmuls xsq^T @ a^2 so the
    max/sqrt/reciprocal chain runs in [128, 16] layout (all DVE lanes
    active); rnorm rows bounce through DRAM and come back as one batched
    broadcast DMA per panel (step-0 partition APs are legal on DRAM).
  - Matmul inputs are bf16 (PE runs f32 at quarter rate); PSUM accumulates
    f32; each panel's 8 PSUM tiles are copied into one SBUF tile and leave
    in a single 2 MB DMA.
  - DMAs are batched aggressively: the SP sequencer pays ~600 ns per
    dma_start, so the panel pipeline uses ~7 DMAs per 512-column panel.
"""

from contextlib import ExitStack

import numpy as np

N, D, H = 8192, 256, 4
NCORES = 8
NLOC = N // NCORES  # 1024 output rows per core
P = 128
PANEL = 512
NPANELS = N // PANEL  # 16
RBLK = NLOC // P  # 8 row blocks of the local output
KCH = (H * D) // P  # 8 contraction chunks of 128
CHD = D // P  # 2 chunks per head
SUB = PANEL // P  # 4 column sub-blocks per panel
EPS = 1e-12

_COMPILED = {}


def _build_bass():
    import concourse.bass as bass
    import concourse.tile as tile
    from concourse import bacc, mybir

    f32 = mybir.dt.float32
    bf16 = mybir.dt.bfloat16

    nc = bacc.Bacc(
        "TRN2",
        target_bir_lowering=False,
        debug=False,
        enable_asserts=False,
        num_devices=NCORES,
    )
    x_t = nc.dram_tensor("x", [N, D], f32, kind="ExternalInput")
    xl_t = nc.dram_tensor("x_local", [NLOC, D], f32, kind="ExternalInput")
    # Host-precomputed functions of attn_vectors (tiny):
    #   w_sq[d, c*4+h]  = attn[h, c*128+d]^2          (bf16, norm matmul rhs)
    #   asq[d, kc]      = 0.25*attn[h, c*128+d]^2     (f32, kc = h*2+c)
    ws_t = nc.dram_tensor("w_sq", [P, CHD * H], bf16, kind="ExternalInput")
    aq_t = nc.dram_tensor("asq", [P, KCH], f32, kind="ExternalInput")
    out_t = nc.dram_tensor("out", [NLOC, N], f32, kind="ExternalOutput")

    x, xl, out = x_t.ap(), xl_t.ap(), out_t.ap()

    with tile.TileContext(nc) as tc, ExitStack() as ctx:
        consts = ctx.enter_context(tc.tile_pool(name="consts", bufs=1))
        loads = ctx.enter_context(tc.tile_pool(name="loads", bufs=6))
        xtp = ctx.enter_context(tc.tile_pool(name="xtp", bufs=1))
        sq = ctx.enter_context(tc.tile_pool(name="sq", bufs=4))
        small = ctx.enter_context(tc.tile_pool(name="small", bufs=3))
        bcp = ctx.enter_context(tc.tile_pool(name="bcp", bufs=3))
        rhsp = ctx.enter_context(tc.tile_pool(name="rhsp", bufs=3))
        outp = ctx.enter_context(tc.tile_pool(name="outp", bufs=2))
        dram = ctx.enter_context(tc.tile_pool(name="dram", bufs=1, space="DRAM"))
        ps_norm = ctx.enter_context(
            tc.tile_pool(name="ps_norm", bufs=2, space="PSUM")
        )
        ps_out = ctx.enter_context(
            tc.tile_pool(name="ps_out", bufs=4, space="PSUM")
        )

        from concourse.masks import make_identity

        w_sq = consts.tile([P, CHD * H], bf16)
        nc.sync.dma_start(w_sq[:], ws_t.ap()[:])
        asq = consts.tile([P, KCH], f32)
        nc.sync.dma_start(asq[:], aq_t.ap()[:])
        ident = consts.tile([P, P], f32)
        make_identity(nc, ident[:])

        def sb_rearr(tile_ap):
            return tile_ap[:].rearrange("q (i d) -> q i d", i=SUB)

        def x_rearr(ap, row0):
            return ap[row0 : row0 + PANEL, :].rearrange(
                "(i q) d -> q i d", q=P
            )

        def prepass(src_ap, row0, xT_tile, name):
            """Load 512 source rows (one batched DMA), transpose them on the
            PE into bf16 x^T, and park this panel's rnorm in DRAM.
            The PSUM->SBUF copy after each transpose doubles as the f32->bf16
            cast."""
            xt = loads.tile([P, SUB * D], f32, tag="xload")
            nc.sync.dma_start(sb_rearr(xt), x_rearr(src_ap, row0))
            for c in range(CHD):
                tp4 = ps_norm.tile([P, PANEL], f32, tag="tp")
                for i in range(SUB):
                    nc.tensor.transpose(
                        tp4[:, i * P : (i + 1) * P],
                        xt[:, i * D + c * P : i * D + (c + 1) * P],
                        ident[:],
                    )
                nc.vector.tensor_copy(
                    xT_tile[:, c * PANEL : (c + 1) * PANEL], tp4[:]
                )
            pn = ps_norm.tile([P, SUB * H], f32, tag="pn")
            xsq = sq.tile([P, CHD * PANEL], bf16, tag="xsq")
            nc.vector.tensor_mul(xsq[:], xT_tile[:], xT_tile[:])
            for i in range(SUB):
                for c in range(CHD):
                    nc.tensor.matmul(
                        pn[:, i * H : (i + 1) * H],
                        xsq[:, c * PANEL + i * P : c * PANEL + (i + 1) * P],
                        w_sq[:, c * H : (c + 1) * H],
                        start=(c == 0),
                        stop=(c == CHD - 1),
                    )
            # eps-clamp; the input AP also permutes [q,(i h)] -> [q,(h i)]
            # so that after the PE transpose the store is contiguous.
            clamped = small.tile([P, SUB * H], f32, tag="clamped")
            nc.vector.tensor_scalar_max(
                clamped[:],
                pn[:].rearrange("q (i h) -> q h i", h=H),
                EPS,
            )
            root = small.tile([P, SUB * H], f32, tag="root")
            nc.scalar.sqrt(root[:], clamped[:])
            rnorm = small.tile([P, SUB * H], f32, tag="rnorm")
            nc.vector.reciprocal(rnorm[:], root[:])
            # [128, 16] -> [16, 128]; row j = h*4+i, so the flat DRAM tile
            # is rnorm_h[i*128+q] at offset h*512 + i*128 + q (h-major).
            pt = ps_norm.tile([SUB * H, P], f32, tag="tp")
            nc.tensor.transpose(pt[:], rnorm[:], ident[:])
            rno = small.tile([SUB * H, P], f32, tag="rno")
            nc.vector.tensor_copy(rno[:], pt[:])
            rnd = dram.tile([SUB * H, P], f32, name=name)
            nc.sync.dma_start(rnd[:], rno[:])
            return rnd

        def bcast_rnorm(rnd):
            """[128, 4*512] f32: bc[:, h*512 + n] = rnorm_h[n], one DMA."""
            bc = bcp.tile([P, H * PANEL], f32, tag="bc")
            src = bass.AP(
                rnd.tensor,
                rnd.offset,
                [[0, P], [PANEL, H], [1, PANEL]],
            )
            nc.sync.dma_start(
                bc[:].rearrange("p (h n) -> p h n", h=H), src
            )
            return bc

        # ---- all prepasses first ------------------------------------------
        # Tile's per-engine instruction order is static, so the lhsT-build
        # DVE ops (which wait on the rnorm DRAM bounce) must come AFTER every
        # prepass op or they head-of-line-block the prepass copies and starve
        # the PE of transpose work during the wait.
        lhsT = consts.tile([P, KCH * NLOC], bf16)
        xlocT = []
        lrnds = []
        for lp in range(2):
            t = consts.tile([P, CHD * PANEL], bf16, name=f"xlocT{lp}")
            xlocT.append(t)
            lrnds.append(prepass(xl, lp * PANEL, t, f"lrnd{lp}"))
        PIPE = 4  # panels of prepass lookahead over the main loop
        xTs = []
        rnds = []

        def prepass_x(p):
            t = xtp.tile([P, CHD * PANEL], bf16, name=f"xT{p}")
            xTs.append(t)
            rnds.append(prepass(x, p * PANEL, t, f"rnd{p}"))

        for p in range(PIPE):
            prepass_x(p)

        # ---- resident lhsT -------------------------------------------------
        for lp in range(2):
            t = xlocT[lp]
            bc = bcast_rnorm(lrnds[lp])
            for h in range(H):
                for c in range(CHD):
                    kc = h * CHD + c
                    scaled = sq.tile([P, PANEL], f32, tag="scaled")
                    nc.vector.tensor_scalar_mul(
                        scaled[:],
                        bc[:, h * PANEL : (h + 1) * PANEL],
                        asq[:, kc : kc + 1],
                    )
                    nc.vector.tensor_mul(
                        lhsT[
                            :,
                            kc * NLOC + lp * PANEL : kc * NLOC + (lp + 1) * PANEL,
                        ],
                        t[:, c * PANEL : (c + 1) * PANEL],
                        scaled[:],
                    )

        # ---- main loop over 16 column panels (prepass pipelined ahead) -----
        for p in range(NPANELS):
            bc = bcast_rnorm(rnds[p])
            # Issue the prepass for panel p+PIPE after this panel's broadcast:
            # its DVE/PE ops fill scheduling gaps without ever blocking the
            # current panel's work (static per-engine order).
            rhs = rhsp.tile([P, KCH * PANEL], bf16, tag="rhs")
            # One batched multiply builds the whole Y'^T panel:
            #   rhs[:, (h*2+c)*512 + n] = xT[:, c*512 + n] * bc[:, h*512 + n]
            xT = xTs[p]
            in0 = bass.AP(
                xT.tensor,
                xT.offset,
                [list(xT.ap[0]), [0, H], [PANEL, CHD], [1, PANEL]],
            )
            in1 = bass.AP(
                bc.tensor,
                bc.offset,
                [list(bc.ap[0]), [PANEL, H], [0, CHD], [1, PANEL]],
            )
            nc.vector.tensor_tensor(
                rhs[:].rearrange("q (h c n) -> q h c n", h=H, c=CHD),
                in0,
                in1,
                mybir.AluOpType.mult,
            )
            if p + PIPE < NPANELS:
                prepass_x(p + PIPE)

            ot = outp.tile([P, RBLK * PANEL], f32, tag="ot")
            for r in range(RBLK):
                acc = ps_out.tile([P, PANEL], f32, tag="acc")
                for kc in range(KCH):
                    nc.tensor.matmul(
                        acc[:],
                        lhsT[:, kc * NLOC + r * P : kc * NLOC + (r + 1) * P],
                        rhs[:, kc * PANEL : (kc + 1) * PANEL],
                        start=(kc == 0),
                        stop=(kc == KCH - 1),
                    )
                nc.vector.tensor_copy(
                    ot[:, r * PANEL : (r + 1) * PANEL], acc[:]
                )
                # Last panel: ship each row block as soon as it is ready so
                # the kernel tail is one small DMA, not copy-all-then-DMA.
                if p == NPANELS - 1:
                    nc.sync.dma_start(
                        out[
                            r * P : (r + 1) * P,
                            p * PANEL : (p + 1) * PANEL,
                        ],
                        ot[:, r * PANEL : (r + 1) * PANEL],
                    )
            if p != NPANELS - 1:
                nc.sync.dma_start(
                    out[:, p * PANEL : (p + 1) * PANEL].rearrange(
                        "(r q) c -> q r c", q=P
                    ),
                    ot[:].rearrange("q (r c) -> q r c", r=RBLK),
                )

    nc.compile()
    return nc


def _get_compiled():
    if "nc" not in _COMPILED:
        _COMPILED["nc"] = _build_bass()
    return _COMPILED["nc"]


def host_side_inputs(x, attn):
    """Per-core input maps (w_sq / asq are tiny host-precomputed functions
    of attn_vectors; see _build_bass)."""
    import ml_dtypes

    w_sq = np.zeros((P, CHD * H), dtype=np.float32)
    asq = np.zeros((P, KCH), dtype=np.float32)
    for c in range(CHD):
        w_sq[:, c * H : (c + 1) * H] = (attn[:, c * P : (c + 1) * P] ** 2).T
    for kc in range(KCH):
        h, c = divmod(kc, CHD)
        asq[:, kc] = 0.25 * attn[h, c * P : (c + 1) * P] ** 2
    w_sq = w_sq.astype(ml_dtypes.bfloat16)
    return [
        {
            "x": x,
            "x_local": np.ascontiguousarray(x[c * NLOC : (c + 1) * NLOC]),
            "w_sq": w_sq,
            "asq": asq,
        }
        for c in range(NCORES)
    ]


def kernel(**inputs) -> np.ndarray:
    from concourse import bass_utils

    x = np.ascontiguousarray(np.asarray(inputs["x"], dtype=np.float32))
    attn = np.ascontiguousarray(
        np.asarray(inputs["attn_vectors"], dtype=np.float32)
    )
    nc = _get_compiled()
    res = bass_utils.run_bass_kernel_spmd(
        nc, host_side_inputs(x, attn), core_ids=list(range(NCORES))
    )
    out = np.concatenate([r["out"] for r in res.results], axis=0)
    # The exact result is symmetric; the bf16 rounding errors of the two
    # triangles are independent, so symmetrizing averages them down.
    return ((out + out.T) * 0.5).astype(np.float32)


# revision 46
# speedup vs baseline: 1.0607x; 1.0456x over previous
"""Self-contained Trainium2 Bass kernel for the "Attentive" GNN message-passing
problem:

    x: [8192, 256] f32, attn_vectors: [4, 256] f32
    e_h = l2_normalize(attn_vectors[h] * x, axis=-1)        # [H, N, D]
    Y   = concat_h(e_h)                                     # [N, H*D]
    out = (Y @ Y.T) / H                                     # [N, N]

Strategy (8 NeuronCores, SPMD, no collectives):
  - Output rows are sharded 8 x 1024; every core receives the FULL x plus its
    own x_local row-shard as separate inputs, so the program is core-agnostic.
  - Key algebra: out[i,j] = sum_k (x*a^2*r/H)[i,k] * (x*r)[j,k] with
    r_h[n] = 1/sqrt(max(sum_d (a_h[d]*x[n,d])^2, eps)); a^2 and the 1/H are
    folded into the (small, resident) lhsT side only, so the streamed rhs
    panels need just one elementwise multiply each.
  - Everything runs in "features on partitions" layout (x^T), obtained by
    staging a bf16 copy of x in DRAM (per-panel tiles, fine-grained deps)
    and reading it back through the DMA xbar transpose.
  - Row norms are computed as transposed PE matmuls xsq^T @ a^2 so the
    max/sqrt/reciprocal chain runs in [128, 16] layout (all DVE lanes
    active); rnorm rows bounce through DRAM and come back as one batched
    broadcast DMA per panel (step-0 partition APs are legal on DRAM).
  - Matmul inputs are bf16 (PE runs f32 at quarter rate); PSUM accumulates
    f32; each panel's 8 PSUM tiles are copied into one SBUF tile and leave
    in a single 2 MB DMA.
  - DMAs are batched aggressively: the SP sequencer pays ~600 ns per
    dma_start, so the panel pipeline uses ~7 DMAs per 512-column panel.
"""

from contextlib import ExitStack

import numpy as np

N, D, H = 8192, 256, 4
NCORES = 8
NLOC = N // NCORES  # 1024 output rows per core
P = 128
PANEL = 512
NPANELS = N // PANEL  # 16
RBLK = NLOC // P  # 8 row blocks of the local output
KCH = (H * D) // P  # 8 contraction chunks of 128
CHD = D // P  # 2 chunks per head
SUB = PANEL // P  # 4 column sub-blocks per panel
EPS = 1e-12

_COMPILED = {}


def _build_bass():
    import concourse.bass as bass
    import concourse.tile as tile
    from concourse import bacc, mybir

    f32 = mybir.dt.float32
    bf16 = mybir.dt.bfloat16

    nc = bacc.Bacc(
        "TRN2",
        target_bir_lowering=False,
        debug=False,
        enable_asserts=False,
        num_devices=NCORES,
    )
    x_t = nc.dram_tensor("x", [N, D], f32, kind="ExternalInput")
    xl_t = nc.dram_tensor("x_local", [NLOC, D], f32, kind="ExternalInput")
    # Host-precomputed functions of attn_vectors (tiny):
    #   w_sq[d, c*4+h]  = attn[h, c*128+d]^2          (bf16, norm matmul rhs)
    #   asq[d, kc]      = 0.25*attn[h, c*128+d]^2     (f32, kc = h*2+c)
    ws_t = nc.dram_tensor("w_sq", [P, CHD * H], bf16, kind="ExternalInput")
    aq_t = nc.dram_tensor("asq", [P, KCH], f32, kind="ExternalInput")
    out_t = nc.dram_tensor("out", [NLOC, N], f32, kind="ExternalOutput")

    x, xl, out = x_t.ap(), xl_t.ap(), out_t.ap()

    with tile.TileContext(nc) as tc, ExitStack() as ctx:
        consts = ctx.enter_context(tc.tile_pool(name="consts", bufs=1))
        loads = ctx.enter_context(tc.tile_pool(name="loads", bufs=6))
        xtp = ctx.enter_context(tc.tile_pool(name="xtp", bufs=1))
        sq = ctx.enter_context(tc.tile_pool(name="sq", bufs=4))
        small = ctx.enter_context(tc.tile_pool(name="small", bufs=3))
        bcp = ctx.enter_context(tc.tile_pool(name="bcp", bufs=3))
        rhsp = ctx.enter_context(tc.tile_pool(name="rhsp", bufs=3))
        outp = ctx.enter_context(tc.tile_pool(name="outp", bufs=2))
        dram = ctx.enter_context(tc.tile_pool(name="dram", bufs=1, space="DRAM"))
        ps_norm = ctx.enter_context(
            tc.tile_pool(name="ps_norm", bufs=2, space="PSUM")
        )
        ps_out = ctx.enter_context(
            tc.tile_pool(name="ps_out", bufs=4, space="PSUM")
        )

        from concourse.masks import make_identity

        w_sq = consts.tile([P, CHD * H], bf16)
        nc.sync.dma_start(w_sq[:], ws_t.ap()[:])
        asq = consts.tile([P, KCH], f32)
        nc.sync.dma_start(asq[:], aq_t.ap()[:])
        ident = consts.tile([P, P], f32)
        make_identity(nc, ident[:])
        identb = consts.tile([P, P], bf16)
        make_identity(nc, identb[:])

        def sb_rearr(tile_ap):
            return tile_ap[:].rearrange("q (i d) -> q i d", i=SUB)

        def x_rearr(ap, row0):
            return ap[row0 : row0 + PANEL, :].rearrange(
                "(i q) d -> q i d", q=P
            )

        def prepass(src_ap, row0, xT_tile, name):
            """Load 512 source rows (one batched DMA), transpose them on the
            PE into bf16 x^T, and park this panel's rnorm in DRAM.
            The PSUM->SBUF copy after each transpose doubles as the f32->bf16
            cast."""
            xt = loads.tile([P, SUB * D], f32, tag="xload")
            nc.sync.dma_start(sb_rearr(xt), x_rearr(src_ap, row0))
            # Round to bf16 before the PE transpose: bf16 streams the PE at
            # 1 cycle/row vs 2 for f32, and the rounding happens exactly once
            # either way (the PSUM->SBUF copy used to do it).
            xtb = loads.tile([P, SUB * D], bf16, tag="xtb")
            nc.vector.tensor_copy(xtb[:], xt[:])
            for c in range(CHD):
                tp4 = ps_norm.tile([P, PANEL], bf16, tag="tp")
                for i in range(SUB):
                    nc.tensor.transpose(
                        tp4[:, i * P : (i + 1) * P],
                        xtb[:, i * D + c * P : i * D + (c + 1) * P],
                        identb[:],
                    )
                nc.vector.tensor_copy(
                    xT_tile[:, c * PANEL : (c + 1) * PANEL], tp4[:]
                )
            pn = ps_norm.tile([P, SUB * H], f32, tag="pn")
            xsq = sq.tile([P, CHD * PANEL], bf16, tag="xsq")
            nc.vector.tensor_mul(xsq[:], xT_tile[:], xT_tile[:])
            for i in range(SUB):
                for c in range(CHD):
                    nc.tensor.matmul(
                        pn[:, i * H : (i + 1) * H],
                        xsq[:, c * PANEL + i * P : c * PANEL + (i + 1) * P],
                        w_sq[:, c * H : (c + 1) * H],
                        start=(c == 0),
                        stop=(c == CHD - 1),
                    )
            # eps-clamp; the input AP also permutes [q,(i h)] -> [q,(h i)]
            # so that after the PE transpose the store is contiguous.
            clamped = small.tile([P, SUB * H], f32, tag="clamped")
            nc.vector.tensor_scalar_max(
                clamped[:],
                pn[:].rearrange("q (i h) -> q h i", h=H),
                EPS,
            )
            root = small.tile([P, SUB * H], f32, tag="root")
            nc.scalar.sqrt(root[:], clamped[:])
            rnorm = small.tile([P, SUB * H], f32, tag="rnorm")
            nc.vector.reciprocal(rnorm[:], root[:])
            # [128, 16] -> [16, 128]; row j = h*4+i, so the flat DRAM tile
            # is rnorm_h[i*128+q] at offset h*512 + i*128 + q (h-major).
            pt = ps_norm.tile([SUB * H, P], f32, tag="tp")
            nc.tensor.transpose(pt[:], rnorm[:], ident[:])
            rno = small.tile([SUB * H, P], f32, tag="rno")
            nc.vector.tensor_copy(rno[:], pt[:])
            rnd = dram.tile([SUB * H, P], f32, name=name)
            nc.sync.dma_start(rnd[:], rno[:])
            return rnd

        def bcast_rnorm(rnd):
            """[128, 4*512] f32: bc[:, h*512 + n] = rnorm_h[n], one DMA."""
            bc = bcp.tile([P, H * PANEL], f32, tag="bc")
            src = bass.AP(
                rnd.tensor,
                rnd.offset,
                [[0, P], [PANEL, H], [1, PANEL]],
            )
            nc.sync.dma_start(
                bc[:].rearrange("p (h n) -> p h n", h=H), src
            )
            return bc

        # ---- all prepasses first ------------------------------------------
        # Tile's per-engine instruction order is static, so the lhsT-build
        # DVE ops (which wait on the rnorm DRAM bounce) must come AFTER every
        # prepass op or they head-of-line-block the prepass copies and starve
        # the PE of transpose work during the wait.
        lhsT = consts.tile([P, KCH * NLOC], bf16)
        xlocT = []
        lrnds = []
        for lp in range(2):
            t = consts.tile([P, CHD * PANEL], bf16, name=f"xlocT{lp}")
            xlocT.append(t)
            lrnds.append(prepass(xl, lp * PANEL, t, f"lrnd{lp}"))
        PIPE = 4  # panels of prepass lookahead over the main loop
        xTs = []
        rnds = []

        def prepass_x(p):
            t = xtp.tile([P, CHD * PANEL], bf16, name=f"xT{p}")
            xTs.append(t)
            rnds.append(prepass(x, p * PANEL, t, f"rnd{p}"))

        for p in range(PIPE):
            prepass_x(p)

        # ---- resident lhsT -------------------------------------------------
        for lp in range(2):
            t = xlocT[lp]
            bc = bcast_rnorm(lrnds[lp])
            for h in range(H):
                for c in range(CHD):
                    kc = h * CHD + c
                    scaled = sq.tile([P, PANEL], f32, tag="scaled")
                    nc.vector.tensor_scalar_mul(
                        scaled[:],
                        bc[:, h * PANEL : (h + 1) * PANEL],
                        asq[:, kc : kc + 1],
                    )
                    nc.vector.tensor_mul(
                        lhsT[
                            :,
                            kc * NLOC + lp * PANEL : kc * NLOC + (lp + 1) * PANEL,
                        ],
                        t[:, c * PANEL : (c + 1) * PANEL],
                        scaled[:],
                    )

        # ---- main loop over 16 column panels (prepass pipelined ahead) -----
        for p in range(NPANELS):
            bc = bcast_rnorm(rnds[p])
            # Issue the prepass for panel p+PIPE after this panel's broadcast:
            # its DVE/PE ops fill scheduling gaps without ever blocking the
            # current panel's work (static per-engine order).
            rhs = rhsp.tile([P, KCH * PANEL], bf16, tag="rhs")
            # One batched multiply builds the whole Y'^T panel:
            #   rhs[:, (h*2+c)*512 + n] = xT[:, c*512 + n] * bc[:, h*512 + n]
            xT = xTs[p]
            in0 = bass.AP(
                xT.tensor,
                xT.offset,
                [list(xT.ap[0]), [0, H], [PANEL, CHD], [1, PANEL]],
            )
            in1 = bass.AP(
                bc.tensor,
                bc.offset,
                [list(bc.ap[0]), [PANEL, H], [0, CHD], [1, PANEL]],
            )
            nc.vector.tensor_tensor(
                rhs[:].rearrange("q (h c n) -> q h c n", h=H, c=CHD),
                in0,
                in1,
                mybir.AluOpType.mult,
            )
            if p + PIPE < NPANELS:
                prepass_x(p + PIPE)

            ot = outp.tile([P, RBLK * PANEL], f32, tag="ot")
            for r in range(RBLK):
                acc = ps_out.tile([P, PANEL], f32, tag="acc")
                for kc in range(KCH):
                    nc.tensor.matmul(
                        acc[:],
                        lhsT[:, kc * NLOC + r * P : kc * NLOC + (r + 1) * P],
                        rhs[:, kc * PANEL : (kc + 1) * PANEL],
                        start=(kc == 0),
                        stop=(kc == KCH - 1),
                    )
                nc.vector.tensor_copy(
                    ot[:, r * PANEL : (r + 1) * PANEL], acc[:]
                )
                # Last panel: ship each row block as soon as it is ready so
                # the kernel tail is one small DMA, not copy-all-then-DMA.
                if p == NPANELS - 1:
                    nc.sync.dma_start(
                        out[
                            r * P : (r + 1) * P,
                            p * PANEL : (p + 1) * PANEL,
                        ],
                        ot[:, r * PANEL : (r + 1) * PANEL],
                    )
            if p != NPANELS - 1:
                nc.sync.dma_start(
                    out[:, p * PANEL : (p + 1) * PANEL].rearrange(
                        "(r q) c -> q r c", q=P
                    ),
                    ot[:].rearrange("q (r c) -> q r c", r=RBLK),
                )

    nc.compile()
    return nc


def _get_compiled():
    if "nc" not in _COMPILED:
        _COMPILED["nc"] = _build_bass()
    return _COMPILED["nc"]


def host_side_inputs(x, attn):
    """Per-core input maps (w_sq / asq are tiny host-precomputed functions
    of attn_vectors; see _build_bass)."""
    import ml_dtypes

    w_sq = np.zeros((P, CHD * H), dtype=np.float32)
    asq = np.zeros((P, KCH), dtype=np.float32)
    for c in range(CHD):
        w_sq[:, c * H : (c + 1) * H] = (attn[:, c * P : (c + 1) * P] ** 2).T
    for kc in range(KCH):
        h, c = divmod(kc, CHD)
        asq[:, kc] = 0.25 * attn[h, c * P : (c + 1) * P] ** 2
    w_sq = w_sq.astype(ml_dtypes.bfloat16)
    return [
        {
            "x": x,
            "x_local": np.ascontiguousarray(x[c * NLOC : (c + 1) * NLOC]),
            "w_sq": w_sq,
            "asq": asq,
        }
        for c in range(NCORES)
    ]


def kernel(**inputs) -> np.ndarray:
    from concourse import bass_utils

    x = np.ascontiguousarray(np.asarray(inputs["x"], dtype=np.float32))
    attn = np.ascontiguousarray(
        np.asarray(inputs["attn_vectors"], dtype=np.float32)
    )
    nc = _get_compiled()
    res = bass_utils.run_bass_kernel_spmd(
        nc, host_side_inputs(x, attn), core_ids=list(range(NCORES))
    )
    out = np.concatenate([r["out"] for r in res.results], axis=0)
    # The exact result is symmetric; the bf16 rounding errors of the two
    # triangles are independent, so symmetrizing averages them down.
    return ((out + out.T) * 0.5).astype(np.float32)
